# revision 1
# baseline (speedup 1.0000x reference)
"""FFF (fast feedforward / tree-MoE routing) Trainium2 kernel.

B=16384 samples route through a depth-12 binary tree (4095 nodes).
Per level: logit = <x[b], W1[node[b]]>, node <- 2*node + 1 + (logit>=0),
out[b] = sum_l gelu(logit_l) * w2s[node_l].

8 cores x 2048 samples (pure data parallel). Per core:
  - levels 0..DENSE-1 "dense": logits for all 2^DENSE-1 shallow nodes in one
    f32 PE matmul (x^T built on-chip via PE transposes); per-level selection
    via one-hot ops on DVE; w2 contribution via (gelu-scaled one-hot) @ w2
    on PE in float32r (4x faster, ~1e-4 rel err).
  - levels DENSE..11 "deep": per-sample row gathers from an interleaved
    HBM table WB[n] = [W1[n] as f32 | w2[n] as bf16] (one indirect DMA per
    128 samples serves both the routing dot and the output accumulate);
    dots and accumulates via fused scalar_tensor_tensor(+accum) on DVE,
    one pass each; node updates in half-batches so the next level's
    gathers overlap the current level's tail.
"""

import numpy as np

P = 128          # partitions
C = 16           # sample chunks per core (P*C = 2048 samples/core)
NCORES = 8
D = 768          # feature dim
KD = 6           # d chunks of 128
DEPTH = 11
NLEV = DEPTH + 1                 # 12 levels
N_NODES = 2 ** NLEV - 1          # 4095
DENSE = 8                        # levels 0..7 dense
NDN = 2 ** DENSE                 # 256 padded dense nodes (0..254 used)
DEEP_LEVELS = list(range(DENSE, NLEV))   # [8, 9, 10, 11]
WB_BYTES = D * 4 + D * 2         # 4608: W1 row f32 | w2 row bf16
N_ADD_GPS = 0                     # deep adds routed via ACT+gpsimd per level

_CACHE = {}


def _build_module():
    import concourse.bacc as bacc
    import concourse.bass as bass
    import concourse.mybir as mybir
    import concourse.tile as tile
    from concourse.masks import make_identity

    f32 = mybir.dt.float32
    f32r = mybir.dt.float32r
    bf16 = mybir.dt.bfloat16
    u8 = mybir.dt.uint8
    i32 = mybir.dt.int32
    i16 = mybir.dt.int16
    Alu = mybir.AluOpType
    Act = mybir.ActivationFunctionType

    nc = bacc.Bacc("TRN2", target_bir_lowering=False, debug=False,
                   num_devices=NCORES)

    x_in = nc.dram_tensor("x_in", [P * C, D], f32, kind="ExternalInput")
    wb = nc.dram_tensor("wb", [N_NODES, WB_BYTES], u8, kind="ExternalInput")
    w1dT = nc.dram_tensor("w1dT", [D, NDN], f32, kind="ExternalInput")
    w2sh = nc.dram_tensor("w2sh", [NDN, D], f32, kind="ExternalInput")
    y_out = nc.dram_tensor("y_out", [P * C, D], f32, kind="ExternalOutput")

    NSPLIT = 8          # x/y DMA chunking (C/NSPLIT chunks each)
    CSP = C // NSPLIT

    with tile.TileContext(nc) as tc:
        with tc.tile_pool(name="persist", bufs=1) as pp:
            identity = pp.tile([P, P], f32)
            make_identity(nc, identity[:])
            iota_i = pp.tile([P, NDN], i32)
            nc.gpsimd.iota(iota_i[:], pattern=[[1, NDN]], base=0,
                           channel_multiplier=0)
            iota_f = pp.tile([P, NDN], f32)
            nc.vector.tensor_copy(iota_f[:], iota_i[:])

            x_sl = pp.tile([P, C, D], f32)
            for s in range(NSPLIT):
                nc.sync.dma_start(
                    out=x_sl[:, s * CSP:(s + 1) * CSP, :],
                    in_=x_in.ap().rearrange("(p c) d -> p c d", p=P)
                        [:, s * CSP:(s + 1) * CSP, :])
            w2sh_sb = pp.tile([P, NDN // P, D], f32r)
            nc.sync.dma_start(
                out=w2sh_sb[:],
                in_=w2sh.ap().rearrange("(nk np) d -> np nk d", np=P)
                    .bitcast(f32r))

            # routing state
            nodes = pp.tile([P, C], f32)
            nc.vector.memset(nodes[:], 0.0)
            logit_sel = pp.tile([P, C], f32)
            bit1 = pp.tile([P, C], f32)
            acc = pp.tile([P, C, D], f32)

            def _nodes_bcast(w, sl, n):
                return nodes[:, sl].rearrange("p (c o) -> p c o", o=1) \
                    .to_broadcast([P, n, w])

            def _iota_bcast(b, w, n):
                return iota_f[:, b:b + w].rearrange("p (o w) -> p o w", o=1) \
                    .to_broadcast([P, n, w])

            # ---------------- phase 1: dense logits (f32) ----------------
            p12 = tc.alloc_tile_pool(name="ph12", bufs=1)
            if True:
                logits_all = p12.tile([P, C, NDN], f32)
                with tc.tile_pool(name="ph1", bufs=1) as p1, \
                     tc.tile_pool(name="xtring", bufs=6) as xtr, \
                     tc.tile_pool(name="ps_t", bufs=3, space="PSUM") as pst, \
                     tc.tile_pool(name="ps_log", bufs=2, space="PSUM") as psl:
                    w1dT_sb = p1.tile([P, KD, NDN], f32)
                    nc.sync.dma_start(
                        out=w1dT_sb[:],
                        in_=w1dT.ap().rearrange("(k p) n -> p k n", p=P))
                    for c in range(C):
                        xts = []
                        for k in range(KD):
                            ps_tr = pst.tile([P, P], f32, tag="pst")
                            nc.tensor.transpose(
                                out=ps_tr[:],
                                in_=x_sl[:, c, k * P:(k + 1) * P],
                                identity=identity[:])
                            xt = xtr.tile([P, P], f32, tag="xt")
                            if k % 2 == 0:
                                nc.scalar.copy(out=xt[:], in_=ps_tr[:])
                            else:
                                nc.vector.tensor_copy(out=xt[:], in_=ps_tr[:])
                            xts.append(xt)
                        ps_log = psl.tile([P, NDN], f32, tag="pslog")
                        for k in range(KD):
                            nc.tensor.matmul(
                                ps_log[:], lhsT=xts[k][:],
                                rhs=w1dT_sb[:, k, :],
                                start=(k == 0), stop=(k == KD - 1))
                        nc.scalar.copy(out=logits_all[:, c, :], in_=ps_log[:])

                # deep-phase pools allocated early: their SBUF space is
                # disjoint from phase-2/3 pools, so level-8 gathers (which
                # only need phase-2 first-half routing) overlap phase 2/3
                dp = tc.alloc_tile_pool(name="deep", bufs=1)
                gp = tc.alloc_tile_pool(name="gbuf", bufs=9)
                pb = tc.alloc_tile_pool(name="pbuf", bufs=3)

                # ---------------- phase 2: shallow routing ----------------
                with tc.tile_pool(name="ph2", bufs=1) as p2, \
                     tc.tile_pool(name="ph2s", bufs=2) as p2s:
                    sonehot = p2.tile([P, C, NDN], f32)
                    nc.vector.memset(sonehot[:], 0.0)
                    HC = C // 2
                    for h2 in (0, 1):
                        sl2 = slice(h2 * HC, (h2 + 1) * HC)
                        for l in range(DENSE):
                            b, w = 2 ** l - 1, 2 ** l
                            oh = sonehot[:, sl2, b:b + w]
                            nc.vector.tensor_tensor(
                                out=oh, in0=_iota_bcast(b, w, HC),
                                in1=_nodes_bcast(w, sl2, HC),
                                op=Alu.is_equal)
                            masked = p2s.tile([P, HC, w], f32,
                                              tag=f"masked{h2}",
                                              name=f"masked{h2}_{l}")
                            lsel = p2s.tile([P, HC], f32, tag=f"lsel{h2}",
                                            name=f"lsel{h2}_{l}")
                            nc.vector.tensor_tensor(
                                out=masked[:], in0=oh,
                                in1=logits_all[:, sl2, b:b + w], op=Alu.mult)
                            nc.vector.tensor_reduce(
                                out=lsel[:], in_=masked[:],
                                axis=mybir.AxisListType.X, op=Alu.add)
                            b1 = p2s.tile([P, HC], f32, tag=f"b1{h2}",
                                          name=f"b1_{h2}_{l}")
                            nc.vector.tensor_scalar(
                                out=b1[:], in0=lsel[:], scalar1=0.0,
                                scalar2=1.0, op0=Alu.is_ge, op1=Alu.add)
                            nc.vector.scalar_tensor_tensor(
                                out=nodes[:, sl2], in0=nodes[:, sl2],
                                scalar=2.0, in1=b1[:], op0=Alu.mult,
                                op1=Alu.add)
                            act_sh = p2s.tile([P, HC], f32,
                                              tag=f"act_sh{h2}",
                                              name=f"act_sh{h2}_{l}")
                            nc.scalar.activation(
                                out=act_sh[:], in_=lsel[:], func=Act.Gelu)
                            act_b = act_sh[:] \
                                .rearrange("p (c o) -> p c o", o=1) \
                                .to_broadcast([P, HC, w])
                            nc.vector.scalar_tensor_tensor(
                                out=oh, in0=oh, scalar=1.0, in1=act_b,
                                op0=Alu.mult, op1=Alu.mult)

                    # ---------- phase 3: shallow w2 matmul (f32r) ----------
                    with tc.tile_pool(name="sonT", bufs=2 * (NDN // P)) as str_, \
                         tc.tile_pool(name="ps_tr3", bufs=3, space="PSUM") as pst3, \
                         tc.tile_pool(name="ps_out", bufs=2, space="PSUM") as pso:
                        for c in range(C):
                            sts = []
                            for t in range(NDN // P):
                                ps_tr = pst3.tile([P, P], f32, tag="pst3")
                                nc.tensor.transpose(
                                    out=ps_tr[:],
                                    in_=sonehot[:, c, t * P:(t + 1) * P],
                                    identity=identity[:])
                                st = str_.tile([P, P], f32r, tag="sonT")
                                nc.scalar.copy(out=st[:], in_=ps_tr[:])
                                sts.append(st)
                            ps_out = pso.tile([P, D], f32, tag="psout")
                            for t in range(NDN // P):
                                nc.tensor.matmul(
                                    ps_out[:, 0:512], lhsT=sts[t][:],
                                    rhs=w2sh_sb[:, t, 0:512],
                                    start=(t == 0), stop=(t == NDN // P - 1))
                                nc.tensor.matmul(
                                    ps_out[:, 512:D], lhsT=sts[t][:],
                                    rhs=w2sh_sb[:, t, 512:D],
                                    start=(t == 0), stop=(t == NDN // P - 1))
                            nc.scalar.copy(out=acc[:, c, :], in_=ps_out[:])

            # ---------------- phase 4: deep levels ----------------
            H = C // 2
            if True:
                act_d = dp.tile([P, C], f32)
                idxs = {}
                for li, l in enumerate(DEEP_LEVELS):
                    for h in (0, 1):
                        sl = slice(h * H, (h + 1) * H)
                        if (l, h) not in idxs:
                            idx0 = dp.tile([P, H], i32, tag=f"idx{h}",
                                           name=f"idx{l}_{h}", bufs=2)
                            nc.vector.tensor_copy(out=idx0[:],
                                                  in_=nodes[:, sl])
                            idxs[(l, h)] = idx0
                        idx = idxs[(l, h)]
                        for cc in range(H):
                            c = h * H + cc
                            g = gp.tile([P, WB_BYTES], u8, tag="g",
                                        name=f"g_{l}_{c}")
                            nc.gpsimd.indirect_dma_start(
                                out=g[:], out_offset=None, in_=wb.ap(),
                                in_offset=bass.IndirectOffsetOnAxis(
                                    ap=idx[:, cc:cc + 1], axis=0))
                            prod = pb.tile([P, D], f32, tag="prod",
                                           name=f"prod{l}_{c}")
                            nc.vector.scalar_tensor_tensor(
                                out=prod[:], in0=x_sl[:, c, :], scalar=1.0,
                                in1=g[:, 0:D * 4].bitcast(f32),
                                op0=Alu.bypass, op1=Alu.mult,
                                accum_out=logit_sel[:, c:c + 1])
                            nc.scalar.activation(
                                out=act_d[:, c:c + 1],
                                in_=logit_sel[:, c:c + 1], func=Act.Gelu)
                            g2 = g[:, D * 4:WB_BYTES].bitcast(bf16)
                            if cc % H < N_ADD_GPS // 2:
                                sc = pb.tile([P, D], f32, tag="sc",
                                             name=f"sc{l}_{c}")
                                nc.gpsimd.tensor_scalar(
                                    out=sc[:], in0=g2,
                                    scalar1=act_d[:, c:c + 1], scalar2=None,
                                    op0=Alu.mult)
                                nc.gpsimd.tensor_tensor(
                                    out=acc[:, c, :], in0=acc[:, c, :],
                                    in1=sc[:], op=Alu.add)
                            else:
                                nc.vector.scalar_tensor_tensor(
                                    out=acc[:, c, :], in0=g2,
                                    scalar=act_d[:, c:c + 1], in1=acc[:, c, :],
                                    op0=Alu.mult, op1=Alu.add)
                        if l != DEEP_LEVELS[-1]:
                            b1h = dp.tile([P, H], f32, tag=f"b1h{h}",
                                          name=f"b1_{l}_{h}", bufs=2)
                            nc.vector.tensor_scalar(
                                out=b1h[:], in0=logit_sel[:, sl], scalar1=0.0,
                                scalar2=1.0, op0=Alu.is_ge, op1=Alu.add)
                            nc.vector.scalar_tensor_tensor(
                                out=nodes[:, sl], in0=nodes[:, sl],
                                scalar=2.0, in1=b1h[:], op0=Alu.mult,
                                op1=Alu.add)
                            nidx = dp.tile([P, H], i32, tag=f"idx{h}",
                                           name=f"idx{l + 1}_{h}", bufs=2)
                            nc.vector.tensor_copy(out=nidx[:],
                                                  in_=nodes[:, sl])
                            idxs[(l + 1, h)] = nidx

            pb.release()
            gp.release()
            dp.release()
            p12.release()
            for s in range(NSPLIT):
                nc.sync.dma_start(
                    out=y_out.ap().rearrange("(p c) d -> p c d", p=P)
                        [:, s * CSP:(s + 1) * CSP, :],
                    in_=acc[:, s * CSP:(s + 1) * CSP, :])

    nc.compile()
    return nc


def _get_module():
    if "nc" not in _CACHE:
        _CACHE["nc"] = _build_module()
    return _CACHE["nc"]


def _make_in_maps(inputs):
    import ml_dtypes
    x = np.asarray(inputs["x"], dtype=np.float32)
    w1s = np.asarray(inputs["w1s"], dtype=np.float32)
    w2s = np.asarray(inputs["w2s"], dtype=np.float32)
    W1 = np.ascontiguousarray(w1s.reshape(N_NODES, D))
    W2 = np.ascontiguousarray(w2s)
    w1dT_np = np.zeros((D, NDN), dtype=np.float32)
    w1dT_np[:, : 2 ** DENSE - 1] = W1[: 2 ** DENSE - 1].T
    w2sh_np = np.zeros((NDN, D), dtype=np.float32)
    w2sh_np[: 2 ** DENSE - 1] = W2[: 2 ** DENSE - 1]
    wb_np = np.concatenate(
        [W1.view(np.uint8).reshape(N_NODES, D * 4),
         W2.astype(ml_dtypes.bfloat16).view(np.uint8).reshape(N_NODES, D * 2)],
        axis=1)
    shard = P * C
    return [{
        "x_in": np.ascontiguousarray(x[k * shard:(k + 1) * shard]),
        "wb": wb_np, "w1dT": w1dT_np, "w2sh": w2sh_np,
    } for k in range(NCORES)]


def kernel(**inputs) -> np.ndarray:
    depth = int(np.asarray(inputs["depth"]))
    assert depth == DEPTH, f"kernel hardcoded for depth=11, got {depth}"
    nc = _get_module()
    from concourse import bass_utils
    res = bass_utils.run_bass_kernel_spmd(
        nc, _make_in_maps(inputs), core_ids=list(range(NCORES)))
    out = np.concatenate([res.results[k]["y_out"] for k in range(NCORES)],
                         axis=0)
    return out.astype(np.float32)


def run_traced(**inputs):
    """Run with NTFF profiling; returns BassKernelResults."""
    from concourse import bass_utils
    nc = _get_module()
    return bass_utils.run_bass_kernel_spmd(
        nc, _make_in_maps(inputs), core_ids=list(range(NCORES)), trace=True)



# revision 48
# speedup vs baseline: 1.6455x; 1.6455x over previous
"""FFF (fast feedforward / tree-MoE routing) Trainium2 kernel, v2.

B=16384 samples route through a depth-12 binary tree (4095 nodes).
Per level: logit = <x[b], W1[node[b]]>, node <- 2*node + 1 + (logit>=0),
out[b] = sum_l gelu(logit_l) * w2s[node_l].

8 cores x 2048 samples (pure data parallel). Per core:
  - levels 0..8 "dense" (512 padded nodes): logits for all nodes via f32r
    PE matmuls from a host-pretransposed xT (no on-chip transposes);
    gelu fused into the PSUM->SBUF evacuation (ACT) so routing bits come
    from sign(gelu(logit)) == sign(logit); fp16 one-hot build/scale on
    DVE (scale pass hits the 2x DVE mode), per-level one-hot reductions
    on gpsimd; processed in two 8-chunk halves so deep gathers overlap.
  - levels 9..11 "deep": 4-chunk-wide indirect gathers of fp16 W1 rows
    (routing-critical, early) and fp16 w2 rows (deferred to the tail);
    routing dots via fused tensor_tensor_reduce on DVE (fp16);
    per-sample w2 scaling on PE via diag(gelu) matmuls.
  - tail: per chunk one PSUM accumulation = transposed scaled one-hot @ w2
    (shallow, 4 ktiles) + 3 diag(gelu) @ gathered-w2 matmuls (deep),
    evacuated once to fp16 and DMA'd out.
"""

import numpy as np

P = 128          # partitions
C = 16           # sample chunks per core (P*C = 2048 samples/core)
HC = C // 2
NCORES = 8
D = 768          # feature dim
KD = 6           # d chunks of 128
DEPTH = 11
NLEV = DEPTH + 1                 # 12 levels
N_NODES = 2 ** NLEV - 1          # 4095
DENSE = 9                        # levels 0..8 dense
NDN = 2 ** DENSE                 # 512 padded dense nodes (0..510 used)
NKT = NDN // P                   # 4 node ktiles
DEEP_LEVELS = [9, 10, 11]
ROW_HALF = D * 2                 # 1536 bytes: one fp16 row
G4 = 4                           # chunks per gather group
NG = C // G4                     # 4 gather groups

_CACHE = {}


def _build_module():
    import concourse.bacc as bacc
    import concourse.bass as bass
    import concourse.mybir as mybir
    import concourse.tile as tile
    from concourse.masks import make_identity

    f32 = mybir.dt.float32
    f32r = mybir.dt.float32r
    fp16 = mybir.dt.float16
    u8 = mybir.dt.uint8
    i32 = mybir.dt.int32
    Alu = mybir.AluOpType
    Act = mybir.ActivationFunctionType

    nc = bacc.Bacc("TRN2", target_bir_lowering=False, debug=False,
                   num_devices=NCORES)

    # host-prepped inputs
    xT_h = nc.dram_tensor("xT_h", [P, KD, C, P], f32, kind="ExternalInput")
    x_h = nc.dram_tensor("x_h", [P * C, D], fp16, kind="ExternalInput")
    w1dT = nc.dram_tensor("w1dT", [D, NDN], f32, kind="ExternalInput")
    w2sh = nc.dram_tensor("w2sh", [NDN, D], fp16, kind="ExternalInput")
    wb1 = nc.dram_tensor("wb1", [N_NODES, ROW_HALF], u8, kind="ExternalInput")
    wb2 = nc.dram_tensor("wb2", [N_NODES, ROW_HALF], u8, kind="ExternalInput")
    wbc = nc.dram_tensor("wbc", [N_NODES, 2 * ROW_HALF], u8,
                         kind="ExternalInput")
    y_out = nc.dram_tensor("y_out", [P * C, D], fp16, kind="ExternalOutput")

    with tile.TileContext(nc) as tc:
        with tc.tile_pool(name="persist", bufs=1) as pp:
            identity = pp.tile([P, P], fp16)
            make_identity(nc, identity[:])
            iota_i = pp.tile([P, NDN], i32)
            nc.gpsimd.iota(iota_i[:], pattern=[[1, NDN]], base=0,
                           channel_multiplier=0)
            iota_f = pp.tile([P, NDN], f32)
            nc.vector.tensor_copy(iota_f[:], iota_i[:])

            # big persistent SBUF tensors
            x_sl = pp.tile([P, C, D], fp16)
            sonehot = pp.tile([P, C, NDN], fp16)
            acc = pp.tile([P, C, D], fp16)
            w2sh_sb = pp.tile([P, NKT, D], fp16)

            # routing state
            nodes = pp.tile([P, C], f32)
            nc.vector.memset(nodes[:], 0.0)
            nc.vector.memset(sonehot[:, :, NDN - 1:NDN], 0.0)
            bitv = pp.tile([P, C], f32)
            b1 = pp.tile([P, C], f32)
            token = pp.tile([P, 3], f32)
            logit_sel = pp.tile([P, C], f32)
            act_d = pp.tile([P, 3, C], f32)
            idxs = {l: pp.tile([P, C], i32, name=f"idx{l}")
                    for l in DEEP_LEVELS}

            # long-lived pools first (pool releases are LIFO)
            gp = tc.alloc_tile_pool(name="w1g", bufs=8)
            prp = tc.alloc_tile_pool(name="prod", bufs=2)
            sTp = tc.alloc_tile_pool(name="sonT", bufs=3)
            ps1p = tc.alloc_tile_pool(name="ps1", bufs=2, space="PSUM")
            pst = tc.alloc_tile_pool(name="ps_tr", bufs=2, space="PSUM")
            psd = tc.alloc_tile_pool(name="ps_deep", bufs=2, space="PSUM")

            # --------- loads ---------
            gelp = tc.alloc_tile_pool(name="gelp", bufs=1)
            gelu_all = gelp.tile([P, C, NDN], fp16)
            ph1 = tc.alloc_tile_pool(name="ph1", bufs=1)
            w1dT_sb = ph1.tile([P, KD, NDN], f32r)
            xT_sb = ph1.tile([P, KD, C, P], f32r)
            w1dT_r = w1dT.ap().rearrange("(k p) n -> p k n", p=P).bitcast(f32r)
            # interleave k-slices of w1dT and xT[q0] so the first phase-1
            # matmuls start as soon as possible
            for k in range(KD):
                nc.sync.dma_start(out=w1dT_sb[:, k:k + 1, :],
                                  in_=w1dT_r[:, k:k + 1, :])
                nc.sync.dma_start(
                    out=xT_sb[:, k:k + 1, 0:G4, :],
                    in_=xT_h.ap()[:, k:k + 1, 0:G4, :].bitcast(f32r))
            for s in range(1, NG):
                nc.sync.dma_start(
                    out=xT_sb[:, :, G4 * s:G4 * s + G4, :],
                    in_=xT_h.ap()[:, :, G4 * s:G4 * s + G4, :].bitcast(f32r))
            nc.sync.dma_start(out=w2sh_sb[:],
                              in_=w2sh.ap().rearrange("(t p) d -> p t d", p=P))
            for s in range(NG):
                nc.sync.dma_start(
                    out=x_sl[:, G4 * s:G4 * s + G4, :],
                    in_=x_h.ap().rearrange("(p c) d -> p c d", p=P)
                        [:, G4 * s:G4 * s + G4, :])

            w1g = {}
            w2g = {}

            def gather_w1(l, q):
                # HW supports one index per partition per indirect DMA
                for c in range(G4 * q, G4 * q + G4):
                    t = gp.tile([P, ROW_HALF], u8, tag="w1g",
                                name=f"w1g_{l}_{c}")
                    nc.gpsimd.indirect_dma_start(
                        out=t[:], out_offset=None, in_=wb1.ap(),
                        in_offset=bass.IndirectOffsetOnAxis(
                            ap=idxs[l][:, c:c + 1], axis=0))
                    w1g[(l, c)] = t[:]

            def gather_c11(q, gcp):
                for c in range(G4 * q, G4 * q + G4):
                    t = gcp.tile([P, 2 * ROW_HALF], u8, tag="c11",
                                 name=f"c11_{c}")
                    nc.gpsimd.indirect_dma_start(
                        out=t[:], out_offset=None, in_=wbc.ap(),
                        in_offset=bass.IndirectOffsetOnAxis(
                            ap=idxs[11][:, c:c + 1], axis=0))
                    w1g[(11, c)] = t[:, 0:ROW_HALF]
                    w2g[(11, c)] = t[:, ROW_HALF:2 * ROW_HALF]

            def gather_w2(l, q, g2p):
                for c in range(G4 * q, G4 * q + G4):
                    t = g2p.tile([P, ROW_HALF], u8, tag="w2g",
                                 name=f"w2g_{l}_{c}")
                    nc.gpsimd.indirect_dma_start(
                        out=t[:], out_offset=None, in_=wb2.ap(),
                        in_offset=bass.IndirectOffsetOnAxis(
                            ap=idxs[l][:, c:c + 1], axis=0))
                    w2g[(l, c)] = t[:]

            # --------- phase 1+2 per quarter; fire L9 W1 gathers asap ------
            def _nodes_bcast(sl, n, w):
                return nodes[:, sl].rearrange("p (c o) -> p c o", o=1) \
                    .to_broadcast([P, n, w])

            def _iota_bcast(b, n, w):
                return iota_f[:, b:b + w].rearrange("p (o w) -> p o w", o=1) \
                    .to_broadcast([P, n, w])

            def phase1(c):
                ps_log = ps1p.tile([P, NDN], f32, tag="pslog",
                                   name=f"pslog{c}")
                for k in range(KD):
                    nc.tensor.matmul(
                        ps_log[:], lhsT=xT_sb[:, k, c, :],
                        rhs=w1dT_sb[:, k, :],
                        start=(k == 0), stop=(k == KD - 1))
                nc.scalar.activation(out=gelu_all[:, c, :],
                                     in_=ps_log[:], func=Act.Gelu)

            def phase2(q):
                sl = slice(q * G4, (q + 1) * G4)
                for l in range(DENSE):
                    b, w = 2 ** l - 1, 2 ** l
                    soh = sonehot[:, sl, b:b + w]
                    gsl = gelu_all[:, sl, b:b + w]
                    if l == 0:
                        nc.vector.tensor_copy(
                            out=soh.rearrange("p c o -> p (c o)"),
                            in_=gsl.rearrange("p c o -> p (c o)"))
                        nc.vector.tensor_scalar(
                            out=b1[:, sl],
                            in0=gsl.rearrange("p c o -> p (c o)"),
                            scalar1=0.0, scalar2=1.0,
                            op0=Alu.is_ge, op1=Alu.add)
                    else:
                        nc.vector.tensor_tensor(
                            out=soh, in0=_iota_bcast(b, G4, w),
                            in1=_nodes_bcast(sl, G4, w), op=Alu.is_equal)
                        nc.vector.tensor_tensor(out=soh, in0=soh, in1=gsl,
                                                op=Alu.mult)
                        nc.vector.tensor_reduce(
                            out=bitv[:, sl], in_=soh,
                            axis=mybir.AxisListType.X, op=Alu.add)
                        nc.vector.tensor_scalar(
                            out=b1[:, sl], in0=bitv[:, sl], scalar1=0.0,
                            scalar2=1.0, op0=Alu.is_ge, op1=Alu.add)
                    nc.vector.scalar_tensor_tensor(
                        out=nodes[:, sl], in0=nodes[:, sl], scalar=2.0,
                        in1=b1[:, sl], op0=Alu.mult, op1=Alu.add)
                nc.vector.tensor_copy(out=idxs[9][:, sl], in_=nodes[:, sl])

            for c in range(C):
                phase1(c)

            # --------- deep: routing dots + one-hot transposes ---------
            def dot(l, c):
                # accum init comes from a zero-valued token tile written after
                # the previous stage completes: pins the scheduler so dots
                # never hoist ahead of routing work whose completion the
                # (SEQ-blocking) gathers wait on
                li = l - 9
                g = w1g[(l, c)]
                prod = prp.tile([P, D], fp16, tag="prod", name=f"pr{l}_{c}")
                nc.vector.scalar_tensor_tensor(
                    out=prod[:], in0=x_sl[:, c, :],
                    scalar=token[:, li:li + 1],
                    in1=g.bitcast(fp16),
                    op0=Alu.add, op1=Alu.mult,
                    accum_out=act_d[:, li, c:c + 1])

            def route_q(l, q):
                sl = slice(q * G4, (q + 1) * G4)
                nc.vector.tensor_scalar(
                    out=b1[:, sl], in0=act_d[:, l - 9, sl], scalar1=0.0,
                    scalar2=1.0, op0=Alu.is_ge, op1=Alu.add)
                nc.vector.scalar_tensor_tensor(
                    out=nodes[:, sl], in0=nodes[:, sl], scalar=2.0,
                    in1=b1[:, sl], op0=Alu.mult, op1=Alu.add)
                nc.vector.tensor_copy(out=idxs[l + 1][:, sl],
                                      in_=nodes[:, sl])

            def trans(c):
                # one-hot transpose, then shallow w2 matmul into acc (fp16)
                pt = pst.tile([P, NDN], fp16, tag="pst", name=f"pt{c}")
                for t in range(NKT):
                    nc.tensor.transpose(
                        out=pt[:, t * P:(t + 1) * P],
                        in_=sonehot[:, c, t * P:(t + 1) * P],
                        identity=identity[:])
                sT = sTp.tile([P, NKT, P], fp16, tag="sonT", name=f"sT{c}")
                nc.scalar.copy(out=sT[:], in_=pt[:])
                po = psd.tile([P, D], f32, tag="psx", name=f"po{c}")
                for t in range(NKT):
                    nc.tensor.matmul(
                        po[:, 0:512], lhsT=sT[:, t, :],
                        rhs=w2sh_sb[:, t, 0:512],
                        start=(t == 0), stop=(t == NKT - 1))
                    nc.tensor.matmul(
                        po[:, 512:D], lhsT=sT[:, t, :],
                        rhs=w2sh_sb[:, t, 512:D],
                        start=(t == 0), stop=(t == NKT - 1))
                nc.scalar.copy(out=acc[:, c, :], in_=po[:])

            def tail_group(g):
                # deep diag matmuls; acc += psum on DVE (idle by now), store
                for c in range(G4 * g, G4 * g + G4):
                    pd = psd.tile([P, D], f32, tag="psx", name=f"pd{c}")
                    nc.tensor.matmul(pd[:, 0:512], lhsT=identity[:],
                                     rhs=acc[:, c, 0:512],
                                     start=True, stop=False)
                    nc.tensor.matmul(pd[:, 512:D], lhsT=identity[:],
                                     rhs=acc[:, c, 512:D],
                                     start=True, stop=False)
                    for li, l in enumerate(DEEP_LEVELS):
                        dgt = dgp.tile([P, P], fp16, tag="diag",
                                       name=f"dg{c}_{li}")
                        nc.scalar.activation(
                            out=dgt[:], in_=identity[:], func=Act.Gelu,
                            scale=act_d[:, li, c:c + 1])
                        g2 = w2g[(l, c)].bitcast(fp16)
                        nc.tensor.matmul(pd[:, 0:512], lhsT=dgt[:],
                                         rhs=g2[:, 0:512], start=False,
                                         stop=(li == 2))
                        nc.tensor.matmul(pd[:, 512:D], lhsT=dgt[:],
                                         rhs=g2[:, 512:D], start=False,
                                         stop=(li == 2))
                    nc.scalar.copy(out=acc[:, c, :], in_=pd[:])
                nc.sync.dma_start(
                    out=y_out.ap().rearrange("(p c) d -> p c d", p=P)
                        [:, G4 * g:G4 * g + G4, :],
                    in_=acc[:, G4 * g:G4 * g + G4, :])

            def dots_q(l, c0):
                for c in range(c0, c0 + G4):
                    dot(l, c)
                    if l == 9:
                        trans(c)

            # interleaved schedule: phase2 quarters, dot quarters per level,
            # quarter-granular routing + gathers, tail groups behind L11 dots
            def set_token(li, src_ap):
                # token[:, li] := 0 * src -- a dependency-only zero
                nc.vector.tensor_scalar(
                    out=token[:, li:li + 1], in0=src_ap,
                    scalar1=0.0, scalar2=0.0, op0=Alu.mult, op1=Alu.add)

            for q in range(NG):
                phase2(q)
                gather_w1(9, q)
            set_token(0, idxs[9][:, C - 1:C])
            # xT / dense-logit buffers are dead now; reuse their SBUF space
            # for the tail pools
            ph1.release()
            gelp.release()
            g2p = tc.alloc_tile_pool(name="w2g", bufs=32)
            gcp = tc.alloc_tile_pool(name="c11", bufs=16)
            dgp = tc.alloc_tile_pool(name="diag", bufs=4)
            for q in range(NG):
                dots_q(9, 4 * q)
                route_q(9, q)
                gather_w1(10, q)
                gather_w2(9, q, g2p)
            set_token(1, idxs[10][:, C - 1:C])
            for q in range(NG):
                dots_q(10, 4 * q)
                route_q(10, q)
                gather_c11(q, gcp)
                gather_w2(10, q, g2p)
            set_token(2, idxs[11][:, C - 1:C])
            for q in range(NG):
                dots_q(11, 4 * q)
                tail_group(q)

            dgp.release()
            gcp.release()
            g2p.release()
            psd.release()
            pst.release()
            ps1p.release()
            sTp.release()
            prp.release()
            gp.release()

    nc.compile()
    return nc


def _get_module():
    if "nc" not in _CACHE:
        _CACHE["nc"] = _build_module()
    return _CACHE["nc"]


def _make_in_maps(inputs):
    x = np.asarray(inputs["x"], dtype=np.float32)
    w1s = np.asarray(inputs["w1s"], dtype=np.float32)
    w2s = np.asarray(inputs["w2s"], dtype=np.float32)
    W1 = np.ascontiguousarray(w1s.reshape(N_NODES, D))
    W2 = np.ascontiguousarray(w2s)

    w1dT_np = np.zeros((D, NDN), dtype=np.float32)
    w1dT_np[:, :NDN - 1] = W1[:NDN - 1].T
    w2sh_np = np.zeros((NDN, D), dtype=np.float16)
    w2sh_np[:NDN - 1] = W2[:NDN - 1].astype(np.float16)
    wb1_np = np.ascontiguousarray(
        W1.astype(np.float16).view(np.uint8).reshape(N_NODES, ROW_HALF))
    wb2_np = np.ascontiguousarray(
        W2.astype(np.float16).view(np.uint8).reshape(N_NODES, ROW_HALF))
    wbc_np = np.ascontiguousarray(np.concatenate([wb1_np, wb2_np], axis=1))

    shard = P * C
    maps = []
    for k in range(NCORES):
        xs = x[k * shard:(k + 1) * shard]                    # [2048, 768]
        # natural layout: sample b = p*C + c
        x_h_np = np.ascontiguousarray(xs.astype(np.float16))
        # xT_h[d, kk, c, m] = xs[m*C + c, kk*128 + d]
        xT = np.ascontiguousarray(
            xs.reshape(P, C, KD, P).transpose(3, 2, 1, 0))
        maps.append({
            "xT_h": xT, "x_h": x_h_np,
            "w1dT": w1dT_np, "w2sh": w2sh_np,
            "wb1": wb1_np, "wb2": wb2_np, "wbc": wbc_np,
        })
    return maps


def kernel(**inputs) -> np.ndarray:
    depth = int(np.asarray(inputs["depth"]))
    assert depth == DEPTH, f"kernel hardcoded for depth=11, got {depth}"
    nc = _get_module()
    from concourse import bass_utils
    res = bass_utils.run_bass_kernel_spmd(
        nc, _make_in_maps(inputs), core_ids=list(range(NCORES)))
    out = np.concatenate([np.asarray(res.results[k]["y_out"])
                          for k in range(NCORES)], axis=0)
    return out.astype(np.float32)


def run_traced(**inputs):
    """Run with NTFF profiling; returns BassKernelResults."""
    from concourse import bass_utils
    nc = _get_module()
    return bass_utils.run_bass_kernel_spmd(
        nc, _make_in_maps(inputs), core_ids=list(range(NCORES)), trace=True)
